# revision 4
# baseline (speedup 1.0000x reference)
"""LGCN encoder (3-layer, dual-adjacency message passing) on 8 Trainium2 cores.

Strategy: 1D row partition of the N=150k node dimension across 8 cores.
Each SpMM is destination-tile structured (128 dst rows per tile):
 - host buckets each core's edges by (tile, matrix, src-window) and packs them
   into a layer-invariant slot stream per (tile-group, window); slots are
   128-edge chunks
 - device fetches source rows with exactly-1024-idx SWDGE dma_gather
   instructions round-robined over 4 SWDGE queues (Q7 prep parallelism)
 - the per-chunk selector S[e, r] = (iota[r]==rowloc[e])*val[e] is built in
   bulk per (group, window) with two broadcast-AP DVE tensor_tensor ops
 - PE accumulates X_g.T @ S into a per-(tile, matrix) PSUM region across all
   windows; the per-layer Linear(2E->E) consumes the two accumulators as lhsT
Between layers an AllGather shares each core's ego shard. Final user/item
lookup is an on-device gather + indirect scatter; host merges by row ownership.
"""
import sys
import os

sys.path.insert(0, "/opt/trn_rl_repo")

import numpy as np
import ml_dtypes
from dataclasses import dataclass

import concourse.bass as bass
import concourse.bacc as bacc
import concourse.mybir as mybir
from concourse.tile import TileContext
from concourse.ap import AP
from concourse import bass_utils

P = 128
E = 128
WIN = 30208          # window rows (int16-addressable with margin)
GI = 1024            # idxs per gather instruction (hard fast-path size)
GC = GI // P         # chunks per gather (8)
NQ = 4               # SWDGE queues
G = 4                # tiles per group (PSUM budget: 2*G accum regions)
NL = 3
DT = mybir.dt.bfloat16


@dataclass(frozen=True)
class Cfg:
    n_users: int
    n_items: int
    n_cores: int
    shard: int
    tiles: int
    n_win: int
    nch_key: tuple     # flattened nch[t][m][w] tuple (structure hash)
    bq: int
    nb: int

    @property
    def shard_g(self):
        return self.tiles * P

    @property
    def n_pad(self):
        return self.shard_g * self.n_cores

    @property
    def n_groups(self):
        return -(-self.tiles // G)

    @property
    def nout(self):
        return 2 * self.nb + 8


def _structure(cfg: Cfg):
    """Static stream layout shared by all cores, derived from nch.

    Returns per (g, w): list of buckets [(t, m, slot0, nslots)], total slots
    S_gw, gather count n_gw; plus global offsets for idx/rl/val arrays.
    """
    nch = np.asarray(cfg.nch_key, np.int64).reshape(cfg.tiles, 2, cfg.n_win)
    groups = []
    for g in range(cfg.n_groups):
        ts = range(g * G, min((g + 1) * G, cfg.tiles))
        per_w = []
        for w in range(cfg.n_win):
            buckets = []
            s = 0
            for t in ts:
                for m in range(2):
                    n = int(nch[t, m, w])
                    buckets.append((t, m, s, n))
                    s += n
            n_gw = -(-s // GC)
            per_w.append((buckets, s, n_gw))
        groups.append(per_w)
    return groups


def build_nc(cfg: Cfg):
    groups = _structure(cfg)
    # per-group idx/scal column extents
    g_idx_cols = [sum(n_gw * (GI // 16) for (_, _, n_gw) in per_w)
                  for per_w in groups]
    g_slot_cols = [sum(S_gw for (_, S_gw, _) in per_w) for per_w in groups]
    idx_cols = sum(g_idx_cols)
    slot_cols = sum(g_slot_cols)
    max_sgw = max(S_gw for per_w in groups for (_, S_gw, _) in per_w)

    nc = bacc.Bacc("TRN2", target_bir_lowering=False, num_swdge_queues=NQ)

    x0 = nc.dram_tensor("x0", [cfg.n_pad, E], DT, kind="ExternalInput")
    gidx = nc.dram_tensor("gidx", [P, idx_cols], mybir.dt.int16,
                          kind="ExternalInput")
    rlv = nc.dram_tensor("rlv", [P, 2, slot_cols], DT, kind="ExternalInput")
    wt = nc.dram_tensor("wt", [P, NL, 2, E], mybir.dt.float32,
                        kind="ExternalInput")
    bb = nc.dram_tensor("bb", [P, NL, E], mybir.dt.float32,
                        kind="ExternalInput")
    iota_in = nc.dram_tensor("iota", [P, P], DT, kind="ExternalInput")
    fidx = nc.dram_tensor("fidx", [P, cfg.bq // 16], mybir.dt.int16,
                          kind="ExternalInput")
    fpos = nc.dram_tensor("fpos", [P, cfg.bq // P], mybir.dt.int32,
                          kind="ExternalInput")
    out_d = nc.dram_tensor("out", [cfg.nout, E], mybir.dt.float32,
                           kind="ExternalOutput")

    rg = [list(range(cfg.n_cores))]
    qrr = [0]

    def next_q():
        q = qrr[0]
        qrr[0] = (q + 1) % NQ
        return q

    with TileContext(nc) as tc:
        with (
            tc.tile_pool(name="const", bufs=1) as constp,
            tc.tile_pool(name="idxp", bufs=3) as idxp,
            tc.tile_pool(name="rlvp", bufs=3) as rlvp,
            tc.tile_pool(name="xgp", bufs=14) as xgp,
            tc.tile_pool(name="svp", bufs=3) as svp,
            tc.tile_pool(name="accp", bufs=6) as accp,
            tc.tile_pool(name="egop", bufs=4) as egop,
            tc.tile_pool(name="ps", bufs=1, space="PSUM") as psp,
            tc.tile_pool(name="dram", bufs=1, space="DRAM") as dramp,
            tc.tile_pool(name="fin", bufs=1) as finp,
        ):
            iota_t = constp.tile([P, P], DT)
            nc.sync.dma_start(out=iota_t[:], in_=iota_in[:, :])
            w_t = constp.tile([P, NL, 2, E], mybir.dt.float32)
            nc.sync.dma_start(out=w_t[:], in_=wt[:, :, :, :])
            b_t = constp.tile([P, NL, E], mybir.dt.float32)
            nc.sync.dma_start(out=b_t[:], in_=bb[:, :, :])

            ego_loc = [
                dramp.tile([cfg.shard_g, E],
                           mybir.dt.float32 if k == NL - 1 else DT,
                           name=f"egoloc{k}")
                for k in range(NL)
            ]
            xsh = [
                dramp.tile([cfg.n_pad, E], DT, addr_space="Shared",
                           name=f"xsh{k}")
                for k in range(NL - 1)
            ]

            for k in range(NL):
                table = x0 if k == 0 else xsh[k - 1]
                idx_off0 = 0
                slot_off0 = 0
                for g, per_w in enumerate(groups):
                    icols = g_idx_cols[g]
                    scols = g_slot_cols[g]
                    idx_t = idxp.tile([P, icols], mybir.dt.int16, tag="idx",
                                      name="idx_t")
                    nc.sync.dma_start(out=idx_t[:],
                                      in_=gidx[:, idx_off0:idx_off0 + icols])
                    rl_t = rlvp.tile([P, 2, scols], DT, tag="rlv", name="rl_t")
                    nc.sync.dma_start(
                        out=rl_t[:],
                        in_=rlv[:, :, slot_off0:slot_off0 + scols])
                    idx_off0 += icols
                    slot_off0 += scols

                    ts = range(g * G, min((g + 1) * G, cfg.tiles))
                    ps = {}
                    for t in ts:
                        for m in range(2):
                            ps[(t, m)] = psp.tile(
                                [P, P], mybir.dt.float32,
                                tag=f"ps{t - g * G}{m}", name=f"ps{m}")
                    io = 0   # idx col offset within group
                    so = 0   # slot col offset within group
                    for w, (buckets, S_gw, n_gw) in enumerate(per_w):
                        if S_gw == 0:
                            continue
                        xgs = []
                        for i in range(n_gw):
                            xg = xgp.tile([P, GC, E], DT, tag="xg", name="xg")
                            nc.gpsimd.dma_gather(
                                xg[:, :, :],
                                table[w * WIN:, :],
                                idx_t[:, io + i * 64: io + (i + 1) * 64],
                                GI, GI, E,
                                queue_num=next_q(),
                            )
                            xgs.append(xg)
                        sv = svp.tile([P, max_sgw, P], DT, tag="sv", name="sv")
                        rl_ap = rl_t[:, 0, so:so + S_gw]
                        vv_ap = rl_t[:, 1, so:so + S_gw]
                        iota_b = AP(iota_t[:, :].tensor, iota_t[:, :].offset,
                                    [list(iota_t[:, :].ap[0]), [0, S_gw],
                                     list(iota_t[:, :].ap[1])])
                        rl_b = AP(rl_ap.tensor, rl_ap.offset,
                                  [list(rl_ap.ap[0]), list(rl_ap.ap[1]),
                                   [0, P]])
                        vv_b = AP(vv_ap.tensor, vv_ap.offset,
                                  [list(vv_ap.ap[0]), list(vv_ap.ap[1]),
                                   [0, P]])
                        svs = sv[:, 0:S_gw, :]
                        nc.vector.tensor_tensor(
                            out=svs, in0=iota_b, in1=rl_b,
                            op=mybir.AluOpType.is_equal)
                        nc.vector.tensor_tensor(
                            out=svs, in0=svs, in1=vv_b,
                            op=mybir.AluOpType.mult)
                        for (t, m, s0, nsl) in buckets:
                            for j in range(nsl):
                                s = s0 + j
                                nc.tensor.matmul(
                                    ps[(t, m)][:],
                                    lhsT=xgs[s // GC][:, s % GC, :],
                                    rhs=sv[:, s, :],
                                    start=(w == 0 and j == 0),
                                    stop=(w == cfg.n_win - 1
                                          and j == nsl - 1),
                                )
                        io += n_gw * 64
                        so += S_gw
                    # linear layer for the group's tiles
                    for t in ts:
                        a0 = accp.tile([P, P], mybir.dt.float32, tag="a0",
                                       name="a0")
                        nc.scalar.copy(out=a0[:], in_=ps[(t, 0)][:])
                        a1 = accp.tile([P, P], mybir.dt.float32, tag="a1",
                                       name="a1")
                        nc.scalar.copy(out=a1[:], in_=ps[(t, 1)][:])
                        eps = psp.tile([P, P], mybir.dt.float32,
                                       tag=f"ps{t - g * G}0", name="eps")
                        nc.tensor.matmul(eps[:], lhsT=a0[:],
                                         rhs=w_t[:, k, 0, :],
                                         start=True, stop=False)
                        nc.tensor.matmul(eps[:], lhsT=a1[:],
                                         rhs=w_t[:, k, 1, :],
                                         start=False, stop=True)
                        odt = mybir.dt.float32 if k == NL - 1 else DT
                        eg = egop.tile([P, P], odt, tag="eg", name="eg")
                        nc.vector.tensor_tensor(
                            out=eg[:], in0=eps[:], in1=b_t[:, k, :],
                            op=mybir.AluOpType.add)
                        nc.sync.dma_start(
                            out=ego_loc[k][t * P:(t + 1) * P, :], in_=eg[:])
                if k < NL - 1:
                    nc.gpsimd.collective_compute(
                        "AllGather",
                        mybir.AluOpType.bypass,
                        replica_groups=rg,
                        ins=[ego_loc[k][:, :]],
                        outs=[xsh[k][:, :]],
                    )

            # final batch lookup: gather rows of ego_loc[-1] then scatter out
            fidx_t = finp.tile([P, cfg.bq // 16], mybir.dt.int16)
            nc.sync.dma_start(out=fidx_t[:], in_=fidx[:, :])
            fpos_t = finp.tile([P, cfg.bq // P], mybir.dt.int32)
            nc.sync.dma_start(out=fpos_t[:], in_=fpos[:, :])
            fg = finp.tile([P, cfg.bq // P, E], mybir.dt.float32)
            for fb in range(0, cfg.bq, GI):
                bs = min(GI, cfg.bq - fb) // P
                nc.gpsimd.dma_gather(
                    fg[:, fb // P:fb // P + bs, :],
                    ego_loc[NL - 1][:, :],
                    fidx_t[:, fb // 16:(fb + bs * P) // 16],
                    bs * P, bs * P, E)
            for j in range(cfg.bq // P):
                nc.gpsimd.indirect_dma_start(
                    out=out_d[:, :],
                    out_offset=bass.IndirectOffsetOnAxis(
                        ap=fpos_t[:, j:j + 1], axis=0),
                    in_=fg[:, j, :],
                    in_offset=None,
                )

    nc.compile()
    return nc


# ---------------------------------------------------------------- host side


def _slot_layout(pj: np.ndarray) -> np.ndarray:
    """pj: [..., J, 128] int16 — idx for gather slot (j, p) of ONE instruction.
    Returns [..., 128, J*8] image (16-partition wrap, 8x replicated)."""
    J = pj.shape[-2]
    v = pj.reshape(*pj.shape[:-2], J, 8, 16)
    nd = v.ndim
    img = v.transpose(*range(nd - 3), nd - 1, nd - 3, nd - 2)
    img = img.reshape(*pj.shape[:-2], 16, J * 8)
    return np.tile(img, (1,) * (img.ndim - 2) + (8, 1))


def make_cfg(inputs, n_users=100000, n_items=50000, n_cores=8):
    n_nodes = n_users + n_items
    shard = n_nodes // n_cores
    tiles = -(-shard // P)
    shard_g = tiles * P
    n_pad = shard_g * n_cores
    n_win = -(-n_pad // WIN)

    adj_row = np.asarray(inputs["adj_row"]).astype(np.int64)
    adj_col = np.asarray(inputs["adj_col"]).astype(np.int64)
    hp_row = np.asarray(inputs["hp_row"]).astype(np.int64)
    hp_col = np.asarray(inputs["hp_col"]).astype(np.int64)

    def gmap(r):
        return (r // shard) * shard_g + (r % shard)

    nch = np.zeros((tiles, 2, n_win), np.int64)
    for m, (row, col) in enumerate(((adj_row, adj_col), (hp_row, hp_col))):
        gr = gmap(row)
        c = gr // shard_g
        t = (gr % shard_g) // P
        w = gmap(col) // WIN
        key = (c * tiles + t) * n_win + w
        cnt = np.bincount(key, minlength=n_cores * tiles * n_win)
        ch = -(-cnt.reshape(n_cores, tiles, n_win) // P)
        nch[:, m, :] = ch.max(axis=0)

    users = np.asarray(inputs["users"]).astype(np.int64)
    items = np.asarray(inputs["items"]).astype(np.int64)
    grow = np.concatenate([users, n_users + items])
    fcnt = np.bincount(grow // shard, minlength=n_cores)
    bq = int(-(-int(fcnt.max()) // P) * P)
    bq = max(bq, P)
    # final gather runs in 1024-idx batches; pad bq so the last batch is full
    bq = int(-(-bq // GI) * GI)

    return Cfg(n_users=n_users, n_items=n_items, n_cores=n_cores, shard=shard,
               tiles=tiles, n_win=n_win, nch_key=tuple(nch.reshape(-1)),
               bq=bq, nb=len(users))


def preprocess(cfg: Cfg, user_emb, item_emb, adj_val, hp_val, W, b,
               adj_row, adj_col, hp_row, hp_col, users, items):
    groups = _structure(cfg)
    nch = np.asarray(cfg.nch_key, np.int64).reshape(cfg.tiles, 2, cfg.n_win)

    # global slot base per (t, m, w) within its (g, w) stream + stream bases
    slot_base = np.zeros((cfg.tiles, 2, cfg.n_win), np.int64)
    # (g, w) -> (slot_col_base, idx_col_base, S_gw, n_gw)
    gw_info = {}
    slot_col = 0
    idx_col = 0
    for g, per_w in enumerate(groups):
        for w, (buckets, S_gw, n_gw) in enumerate(per_w):
            gw_info[(g, w)] = (slot_col, idx_col, S_gw, n_gw)
            for (t, m, s0, nsl) in buckets:
                slot_base[t, m, w] = slot_col + s0
            slot_col += S_gw
            idx_col += n_gw * 64
    total_slots = slot_col
    total_idx_cols = idx_col

    def gmap(r):
        return (r // cfg.shard) * cfg.shard_g + (r % cfg.shard)

    ego0 = np.concatenate([np.asarray(user_emb), np.asarray(item_emb)], axis=0)
    x0 = np.zeros((cfg.n_pad, E), np.float32)
    x0[gmap(np.arange(ego0.shape[0]))] = ego0
    x0 = x0.astype(ml_dtypes.bfloat16)

    mats = [
        (np.asarray(adj_row).astype(np.int64),
         np.asarray(adj_col).astype(np.int64),
         np.asarray(adj_val).astype(np.float32)),
        (np.asarray(hp_row).astype(np.int64),
         np.asarray(hp_col).astype(np.int64),
         np.asarray(hp_val).astype(np.float32)),
    ]

    in_maps = []
    # shared tensors
    Wn = np.asarray(W).astype(np.float32)
    wt = np.stack([Wn[:, :P, :], Wn[:, P:, :]], axis=1).transpose(2, 0, 1, 3)
    wt = np.ascontiguousarray(wt)
    bn = np.asarray(b).astype(np.float32)
    bbn = np.ascontiguousarray(
        np.broadcast_to(bn[None, :, :], (P, NL, E)).astype(np.float32))
    iota = np.ascontiguousarray(
        np.broadcast_to(np.arange(P, dtype=ml_dtypes.bfloat16), (P, P)))

    users = np.asarray(users).astype(np.int64)
    items = np.asarray(items).astype(np.int64)
    grow = np.concatenate([users, cfg.n_users + items])
    pos = np.arange(grow.size)
    fowner = grow // cfg.shard
    aux = dict(fowner=fowner)

    for c in range(cfg.n_cores):
        # slot-stream arrays for this core
        slot_idx = np.zeros((total_slots, P), np.int16)
        slot_rl = np.zeros((total_slots, P), ml_dtypes.bfloat16)
        slot_v = np.zeros((total_slots, P), ml_dtypes.bfloat16)
        for m, (row, col, val) in enumerate(mats):
            gr = gmap(row)
            sel = (gr // cfg.shard_g) == c
            grs = gr[sel] % cfg.shard_g
            t = grs // P
            rl = (grs % P).astype(np.float32)
            gcol = gmap(col[sel])
            w = gcol // WIN
            idx16 = (gcol - w * WIN).astype(np.int16)
            v = val[sel]

            bucket = t * cfg.n_win + w
            order = np.argsort(bucket, kind="stable")
            bs = bucket[order]
            cnts = np.bincount(bs, minlength=cfg.tiles * cfg.n_win)
            starts = np.zeros_like(cnts)
            starts[1:] = np.cumsum(cnts)[:-1]
            rank = np.arange(bs.size) - starts[bs]
            tt = bs // cfg.n_win
            ww = bs % cfg.n_win
            gslot = slot_base[tt, m, ww] + rank // P
            gpart = rank % P
            slot_idx[gslot, gpart] = idx16[order]
            slot_rl[gslot, gpart] = rl[order].astype(ml_dtypes.bfloat16)
            slot_v[gslot, gpart] = v[order].astype(ml_dtypes.bfloat16)

        # build gather idx image: per (g, w): slots padded to n_gw*8
        gidx = np.zeros((P, total_idx_cols), np.int16)
        rlv = np.zeros((P, 2, total_slots), ml_dtypes.bfloat16)
        for (g, w), (sc, ic, S_gw, n_gw) in gw_info.items():
            if S_gw == 0:
                continue
            blk = np.zeros((n_gw * GC, P), np.int16)
            blk[:S_gw] = slot_idx[sc:sc + S_gw]
            img = _slot_layout(blk.reshape(n_gw, GC, P))     # [n_gw, 128, 64]
            gidx[:, ic:ic + n_gw * 64] = img.transpose(1, 0, 2).reshape(
                P, n_gw * 64)
        rlv[:, 0, :] = slot_rl.T
        rlv[:, 1, :] = slot_v.T

        # final lookup
        sel = fowner == c
        lrow = (grow[sel] - c * cfg.shard)
        lrow = (lrow // P) * P + (lrow % P)  # local row in shard (pre-pad)
        # map to padded local position
        lrow = np.asarray(grow[sel] - c * cfg.shard, np.int64)
        ppos = pos[sel].astype(np.int32)
        cnt = lrow.size
        if cnt > cfg.bq:
            raise ValueError(f"bq too small: {cnt}")
        li = np.zeros(cfg.bq, np.int16)
        lp = np.full(cfg.bq, 2 * cfg.nb, np.int32) + np.arange(cfg.bq) % 8
        li[:cnt] = lrow.astype(np.int16)
        lp[:cnt] = ppos
        fimg = [_slot_layout(q.reshape(GC, P))
                for q in li.reshape(-1, GI)]
        fidx = np.concatenate(fimg, axis=1)
        fpos = lp.reshape(cfg.bq // P, P).T.copy()

        in_maps.append(dict(
            x0=x0, gidx=np.ascontiguousarray(gidx),
            rlv=np.ascontiguousarray(rlv),
            wt=wt, bb=bbn, iota=iota,
            fidx=np.ascontiguousarray(fidx),
            fpos=np.ascontiguousarray(fpos),
        ))
    return in_maps, aux


def postprocess(cfg: Cfg, results, aux):
    acc = np.zeros((cfg.nout, E), np.float32)
    fowner = aux["fowner"]
    for c, r in enumerate(results):
        sel = fowner == c
        acc[:2 * cfg.nb][sel] = r["out"][:2 * cfg.nb][sel]
    return acc[:cfg.nb].copy(), acc[cfg.nb:2 * cfg.nb].copy()


_CACHE = {}


def _get_nc(cfg: Cfg):
    if cfg not in _CACHE:
        _CACHE[cfg] = build_nc(cfg)
    return _CACHE[cfg]


def run(cfg, inputs, trace=False):
    nc = _get_nc(cfg)
    in_maps, aux = preprocess(cfg, **inputs)
    res = bass_utils.run_bass_kernel_spmd(
        nc, in_maps, core_ids=list(range(cfg.n_cores)), trace=trace)
    user_out, item_out = postprocess(cfg, res.results, aux)
    return (user_out, item_out), res


def kernel(user_emb, item_emb, adj_val, hp_val, W, b,
           adj_row, adj_col, hp_row, hp_col, users, items):
    inputs = dict(
        user_emb=user_emb, item_emb=item_emb, adj_val=adj_val, hp_val=hp_val,
        W=W, b=b,
        adj_row=np.asarray(adj_row).astype(np.int64),
        adj_col=np.asarray(adj_col).astype(np.int64),
        hp_row=np.asarray(hp_row).astype(np.int64),
        hp_col=np.asarray(hp_col).astype(np.int64),
        users=np.asarray(users).astype(np.int64),
        items=np.asarray(items).astype(np.int64),
    )
    cfg = make_cfg(inputs)
    (user_out, item_out), _ = run(cfg, inputs)
    return user_out, item_out


# revision 10
# speedup vs baseline: 1.1598x; 1.1598x over previous
"""LGCN encoder (3-layer, dual-adjacency message passing) on 8 Trainium2 cores.

Strategy: 1D row partition of the N=150k node dimension across 8 cores.
Each SpMM is destination-tile structured (128 dst rows per tile):
 - host buckets each core's edges by (tile, matrix, src-window) and packs them
   into a layer-invariant slot stream per (tile-group, window); slots are
   128-edge chunks
 - device fetches source rows with exactly-1024-idx SWDGE dma_gather
   instructions round-robined over 4 SWDGE queues (Q7 prep parallelism)
 - the per-chunk selector S[e, r] = (iota[r]==rowloc[e])*val[e] is built in
   bulk per (group, window) with two broadcast-AP DVE tensor_tensor ops
 - PE accumulates X_g.T @ S into a per-(tile, matrix) PSUM region across all
   windows; the per-layer Linear(2E->E) consumes the two accumulators as lhsT
Between layers an AllGather shares each core's ego shard. Final user/item
lookup is an on-device gather + indirect scatter; host merges by row ownership.
"""
import sys
import os

sys.path.insert(0, "/opt/trn_rl_repo")

import numpy as np
import ml_dtypes
from dataclasses import dataclass

import concourse.bass as bass
import concourse.bacc as bacc
import concourse.mybir as mybir
from concourse.tile import TileContext
from concourse.ap import AP
from concourse import bass_utils

P = 128
E = 128
WIN = 30208          # window rows (int16-addressable with margin)
GI = 1024            # idxs per gather instruction (hard fast-path size)
GC = GI // P         # chunks per gather (8)
NQ = 4               # SWDGE queues
G = 4                # tiles per group (PSUM budget: 2*G accum regions)
NL = 3
DT = mybir.dt.bfloat16


@dataclass(frozen=True)
class Cfg:
    n_users: int
    n_items: int
    n_cores: int
    shard: int
    tiles: int
    n_win: int
    nch_key: tuple     # flattened nch[t][m][w] tuple (structure hash)
    bq: int
    nb: int

    @property
    def shard_g(self):
        return self.tiles * P

    @property
    def n_pad(self):
        return self.shard_g * self.n_cores

    @property
    def n_groups(self):
        return -(-self.tiles // G)

    @property
    def nout(self):
        return 2 * self.nb + 8


def _structure(cfg: Cfg):
    """Static stream layout shared by all cores, derived from nch.

    Slot streams are per (layer-invariant) window, spanning all tile groups
    in order; gathers are 1024-idx units of the stream and may straddle a
    group boundary. Returns:
      - gw[(g, w)] = dict(buckets=[(t, m, slot0, nslots)] (slot0 global in
        the w-stream), a=first slot, b=end slot, e0/e1=gather index range
        emitted by this group)
      - S_w[w] = total slots of window w's stream
      - n_gw_tot[w] = ceil(S_w / GC) gathers per window
    """
    nch = np.asarray(cfg.nch_key, np.int64).reshape(cfg.tiles, 2, cfg.n_win)
    gw = {}
    S_w = [0] * cfg.n_win
    for w in range(cfg.n_win):
        s = 0
        e_prev = 0
        for g in range(cfg.n_groups):
            ts = range(g * G, min((g + 1) * G, cfg.tiles))
            a = s
            buckets = []
            for t in ts:
                for m in range(2):
                    n = int(nch[t, m, w])
                    buckets.append((t, m, s, n))
                    s += n
            e1 = -(-s // GC)
            gw[(g, w)] = dict(buckets=buckets, a=a, b=s, e0=e_prev, e1=e1)
            e_prev = e1
        S_w[w] = s
    n_gw_tot = [-(-S_w[w] // GC) for w in range(cfg.n_win)]
    return gw, S_w, n_gw_tot


def build_nc(cfg: Cfg):
    gw, S_w, n_gw_tot = _structure(cfg)
    # per-group idx/scal column extents
    g_idx_cols = [
        sum((gw[(g, w)]["e1"] - gw[(g, w)]["e0"]) * 64
            for w in range(cfg.n_win))
        for g in range(cfg.n_groups)
    ]
    g_slot_cols = [
        sum(gw[(g, w)]["b"] - gw[(g, w)]["a"] for w in range(cfg.n_win))
        for g in range(cfg.n_groups)
    ]
    idx_cols = sum(g_idx_cols)
    slot_cols = sum(g_slot_cols)
    max_sgw = max(gw[(g, w)]["b"] - gw[(g, w)]["a"]
                  for g in range(cfg.n_groups) for w in range(cfg.n_win))

    nc = bacc.Bacc("TRN2", target_bir_lowering=False, num_swdge_queues=NQ)

    x0 = nc.dram_tensor("x0", [cfg.n_pad, E], DT, kind="ExternalInput")
    gidx = nc.dram_tensor("gidx", [P, idx_cols], mybir.dt.int16,
                          kind="ExternalInput")
    rlv = nc.dram_tensor("rlv", [P, 2, slot_cols], DT, kind="ExternalInput")
    wt = nc.dram_tensor("wt", [P, NL, 2, E], mybir.dt.float32,
                        kind="ExternalInput")
    bb = nc.dram_tensor("bb", [P, NL, E], mybir.dt.float32,
                        kind="ExternalInput")
    iota_in = nc.dram_tensor("iota", [P, P], DT, kind="ExternalInput")
    fidx = nc.dram_tensor("fidx", [P, cfg.bq // 16], mybir.dt.int16,
                          kind="ExternalInput")
    fpos = nc.dram_tensor("fpos", [P, cfg.bq // P], mybir.dt.int32,
                          kind="ExternalInput")
    out_d = nc.dram_tensor("out", [cfg.nout, E], mybir.dt.float32,
                           kind="ExternalOutput")

    rg = [list(range(cfg.n_cores))]
    qrr = [0]

    def next_q():
        q = qrr[0]
        qrr[0] = (q + 1) % NQ
        return q

    with TileContext(nc) as tc:
        with (
            tc.tile_pool(name="const", bufs=1) as constp,
            tc.tile_pool(name="idxp", bufs=3) as idxp,
            tc.tile_pool(name="rlvp", bufs=3) as rlvp,
            tc.tile_pool(name="xgp", bufs=24) as xgp,
            tc.tile_pool(name="svp", bufs=3) as svp,
            tc.tile_pool(name="accp", bufs=6) as accp,
            tc.tile_pool(name="egop", bufs=4) as egop,
            tc.tile_pool(name="ps", bufs=1, space="PSUM") as psp,
            tc.tile_pool(name="dram", bufs=1, space="DRAM") as dramp,
            tc.tile_pool(name="fin", bufs=1) as finp,
        ):
            iota_t = constp.tile([P, P], DT)
            nc.sync.dma_start(out=iota_t[:], in_=iota_in[:, :])
            w_t = constp.tile([P, NL, 2, E], mybir.dt.float32)
            nc.sync.dma_start(out=w_t[:], in_=wt[:, :, :, :])
            b_t = constp.tile([P, NL, E], mybir.dt.float32)
            nc.sync.dma_start(out=b_t[:], in_=bb[:, :, :])

            ego_loc = [
                dramp.tile([cfg.shard_g, E],
                           mybir.dt.float32 if k == NL - 1 else DT,
                           name=f"egoloc{k}")
                for k in range(NL)
            ]
            xsh = [
                dramp.tile([cfg.n_pad, E], DT, addr_space="Shared",
                           name=f"xsh{k}")
                for k in range(NL - 1)
            ]

            for k in range(NL):
                table = x0 if k == 0 else xsh[k - 1]
                idx_off0 = 0
                slot_off0 = 0
                gtiles = [dict() for _ in range(cfg.n_win)]
                for g in range(cfg.n_groups):
                    icols = g_idx_cols[g]
                    scols = g_slot_cols[g]
                    idx_t = idxp.tile([P, icols], mybir.dt.int16, tag="idx",
                                      name="idx_t")
                    nc.sync.dma_start(out=idx_t[:],
                                      in_=gidx[:, idx_off0:idx_off0 + icols])
                    rl_t = rlvp.tile([P, 2, scols], DT, tag="rlv", name="rl_t")
                    nc.sync.dma_start(
                        out=rl_t[:],
                        in_=rlv[:, :, slot_off0:slot_off0 + scols])
                    idx_off0 += icols
                    slot_off0 += scols

                    ts = range(g * G, min((g + 1) * G, cfg.tiles))
                    ps = {}
                    for t in ts:
                        for m in range(2):
                            ps[(t, m)] = psp.tile(
                                [P, P], mybir.dt.float32,
                                tag=f"ps{t - g * G}{m}", name=f"ps{m}")
                    io = 0   # idx col offset within group
                    so = 0   # slot col offset within group
                    for w in range(cfg.n_win):
                        info = gw[(g, w)]
                        a, b = info["a"], info["b"]
                        e0, e1 = info["e0"], info["e1"]
                        S_gw = b - a
                        if S_gw == 0:
                            continue
                        # selector build first (DVE runs ahead of gathers)
                        sv = svp.tile([P, max_sgw, P], DT, tag="sv", name="sv")
                        rl_ap = rl_t[:, 0, so:so + S_gw]
                        vv_ap = rl_t[:, 1, so:so + S_gw]
                        iota_b = AP(iota_t[:, :].tensor, iota_t[:, :].offset,
                                    [list(iota_t[:, :].ap[0]), [0, S_gw],
                                     list(iota_t[:, :].ap[1])])
                        rl_b = AP(rl_ap.tensor, rl_ap.offset,
                                  [list(rl_ap.ap[0]), list(rl_ap.ap[1]),
                                   [0, P]])
                        vv_b = AP(vv_ap.tensor, vv_ap.offset,
                                  [list(vv_ap.ap[0]), list(vv_ap.ap[1]),
                                   [0, P]])
                        svs = sv[:, 0:S_gw, :]
                        nc.vector.tensor_tensor(
                            out=svs, in0=iota_b, in1=rl_b,
                            op=mybir.AluOpType.is_equal)
                        nc.vector.tensor_tensor(
                            out=svs, in0=svs, in1=vv_b,
                            op=mybir.AluOpType.mult)
                        for i in range(e0, e1):
                            xg = xgp.tile([P, GC, E], DT, tag="xg", name="xg")
                            nc.gpsimd.dma_gather(
                                xg[:, :, :],
                                table[w * WIN:, :],
                                idx_t[:, io + (i - e0) * 64:
                                      io + (i - e0 + 1) * 64],
                                GI, GI, E,
                                queue_num=next_q(),
                            )
                            gtiles[w][i] = xg
                        for (t, m, s0, nsl) in info["buckets"]:
                            for j in range(nsl):
                                s = s0 + j
                                nc.tensor.matmul(
                                    ps[(t, m)][:],
                                    lhsT=gtiles[w][s // GC][:, s % GC, :],
                                    rhs=sv[:, s - a, :],
                                    start=(w == 0 and j == 0),
                                    stop=(w == cfg.n_win - 1
                                          and j == nsl - 1),
                                )
                        io += (e1 - e0) * 64
                        so += S_gw
                    # linear layer for the group's tiles
                    for t in ts:
                        a0 = accp.tile([P, P], mybir.dt.float32, tag="a0",
                                       name="a0")
                        nc.scalar.copy(out=a0[:], in_=ps[(t, 0)][:])
                        a1 = accp.tile([P, P], mybir.dt.float32, tag="a1",
                                       name="a1")
                        nc.scalar.copy(out=a1[:], in_=ps[(t, 1)][:])
                        eps = psp.tile([P, P], mybir.dt.float32,
                                       tag=f"ps{t - g * G}0", name="eps")
                        nc.tensor.matmul(eps[:], lhsT=a0[:],
                                         rhs=w_t[:, k, 0, :],
                                         start=True, stop=False)
                        nc.tensor.matmul(eps[:], lhsT=a1[:],
                                         rhs=w_t[:, k, 1, :],
                                         start=False, stop=True)
                        odt = mybir.dt.float32 if k == NL - 1 else DT
                        eg = egop.tile([P, P], odt, tag="eg", name="eg")
                        nc.vector.tensor_tensor(
                            out=eg[:], in0=eps[:], in1=b_t[:, k, :],
                            op=mybir.AluOpType.add)
                        nc.sync.dma_start(
                            out=ego_loc[k][t * P:(t + 1) * P, :], in_=eg[:])
                if k < NL - 1:
                    nc.gpsimd.collective_compute(
                        "AllGather",
                        mybir.AluOpType.bypass,
                        replica_groups=rg,
                        ins=[ego_loc[k][:, :]],
                        outs=[xsh[k][:, :]],
                    )

            # final batch lookup: gather rows of ego_loc[-1] then scatter out
            fidx_t = finp.tile([P, cfg.bq // 16], mybir.dt.int16)
            nc.sync.dma_start(out=fidx_t[:], in_=fidx[:, :])
            fpos_t = finp.tile([P, cfg.bq // P], mybir.dt.int32)
            nc.sync.dma_start(out=fpos_t[:], in_=fpos[:, :])
            fg = finp.tile([P, cfg.bq // P, E], mybir.dt.float32)
            for fb in range(0, cfg.bq, GI):
                bs = min(GI, cfg.bq - fb) // P
                nc.gpsimd.dma_gather(
                    fg[:, fb // P:fb // P + bs, :],
                    ego_loc[NL - 1][:, :],
                    fidx_t[:, fb // 16:(fb + bs * P) // 16],
                    bs * P, bs * P, E)
            for j in range(cfg.bq // P):
                nc.gpsimd.indirect_dma_start(
                    out=out_d[:, :],
                    out_offset=bass.IndirectOffsetOnAxis(
                        ap=fpos_t[:, j:j + 1], axis=0),
                    in_=fg[:, j, :],
                    in_offset=None,
                )

    nc.compile()
    return nc


# ---------------------------------------------------------------- host side


def _slot_layout(pj: np.ndarray) -> np.ndarray:
    """pj: [..., J, 128] int16 — idx for gather slot (j, p) of ONE instruction.
    Returns [..., 128, J*8] image (16-partition wrap, 8x replicated)."""
    J = pj.shape[-2]
    v = pj.reshape(*pj.shape[:-2], J, 8, 16)
    nd = v.ndim
    img = v.transpose(*range(nd - 3), nd - 1, nd - 3, nd - 2)
    img = img.reshape(*pj.shape[:-2], 16, J * 8)
    return np.tile(img, (1,) * (img.ndim - 2) + (8, 1))


def make_cfg(inputs, n_users=100000, n_items=50000, n_cores=8):
    n_nodes = n_users + n_items
    shard = n_nodes // n_cores
    tiles = -(-shard // P)
    shard_g = tiles * P
    n_pad = shard_g * n_cores
    n_win = -(-n_pad // WIN)

    adj_row = np.asarray(inputs["adj_row"]).astype(np.int64)
    adj_col = np.asarray(inputs["adj_col"]).astype(np.int64)
    hp_row = np.asarray(inputs["hp_row"]).astype(np.int64)
    hp_col = np.asarray(inputs["hp_col"]).astype(np.int64)

    def gmap(r):
        return (r // shard) * shard_g + (r % shard)

    nch = np.zeros((tiles, 2, n_win), np.int64)
    for m, (row, col) in enumerate(((adj_row, adj_col), (hp_row, hp_col))):
        gr = gmap(row)
        c = gr // shard_g
        t = (gr % shard_g) // P
        w = gmap(col) // WIN
        key = (c * tiles + t) * n_win + w
        cnt = np.bincount(key, minlength=n_cores * tiles * n_win)
        ch = -(-cnt.reshape(n_cores, tiles, n_win) // P)
        nch[:, m, :] = ch.max(axis=0)

    users = np.asarray(inputs["users"]).astype(np.int64)
    items = np.asarray(inputs["items"]).astype(np.int64)
    grow = np.concatenate([users, n_users + items])
    fcnt = np.bincount(grow // shard, minlength=n_cores)
    bq = int(-(-int(fcnt.max()) // P) * P)
    bq = max(bq, P)
    # final gather runs in 1024-idx batches; pad bq so the last batch is full
    bq = int(-(-bq // GI) * GI)

    return Cfg(n_users=n_users, n_items=n_items, n_cores=n_cores, shard=shard,
               tiles=tiles, n_win=n_win, nch_key=tuple(nch.reshape(-1)),
               bq=bq, nb=len(users))


def preprocess(cfg: Cfg, user_emb, item_emb, adj_val, hp_val, W, b,
               adj_row, adj_col, hp_row, hp_col, users, items):
    gw, S_w, n_gw_tot = _structure(cfg)

    # slot base per (t, m, w) within window w's global stream
    slot_base = np.zeros((cfg.tiles, 2, cfg.n_win), np.int64)
    for (g, w), info in gw.items():
        for (t, m, s0, nsl) in info["buckets"]:
            slot_base[t, m, w] = s0
    total_idx_cols = sum(
        (gw[(g, w)]["e1"] - gw[(g, w)]["e0"]) * 64
        for g in range(cfg.n_groups) for w in range(cfg.n_win))
    total_slots = sum(
        gw[(g, w)]["b"] - gw[(g, w)]["a"]
        for g in range(cfg.n_groups) for w in range(cfg.n_win))

    def gmap(r):
        return (r // cfg.shard) * cfg.shard_g + (r % cfg.shard)

    ego0 = np.concatenate([np.asarray(user_emb), np.asarray(item_emb)], axis=0)
    x0 = np.zeros((cfg.n_pad, E), np.float32)
    x0[gmap(np.arange(ego0.shape[0]))] = ego0
    x0 = x0.astype(ml_dtypes.bfloat16)

    mats = [
        (np.asarray(adj_row).astype(np.int64),
         np.asarray(adj_col).astype(np.int64),
         np.asarray(adj_val).astype(np.float32)),
        (np.asarray(hp_row).astype(np.int64),
         np.asarray(hp_col).astype(np.int64),
         np.asarray(hp_val).astype(np.float32)),
    ]

    in_maps = []
    # shared tensors
    Wn = np.asarray(W).astype(np.float32)
    wt = np.stack([Wn[:, :P, :], Wn[:, P:, :]], axis=1).transpose(2, 0, 1, 3)
    wt = np.ascontiguousarray(wt)
    bn = np.asarray(b).astype(np.float32)
    bbn = np.ascontiguousarray(
        np.broadcast_to(bn[None, :, :], (P, NL, E)).astype(np.float32))
    iota = np.ascontiguousarray(
        np.broadcast_to(np.arange(P, dtype=ml_dtypes.bfloat16), (P, P)))

    users = np.asarray(users).astype(np.int64)
    items = np.asarray(items).astype(np.int64)
    grow = np.concatenate([users, cfg.n_users + items])
    pos = np.arange(grow.size)
    fowner = grow // cfg.shard
    aux = dict(fowner=fowner)

    for c in range(cfg.n_cores):
        # per-window slot-stream arrays for this core
        slot_idx = [np.zeros((S_w[w], P), np.int16) for w in range(cfg.n_win)]
        slot_rl = [np.zeros((S_w[w], P), ml_dtypes.bfloat16)
                   for w in range(cfg.n_win)]
        slot_v = [np.zeros((S_w[w], P), ml_dtypes.bfloat16)
                  for w in range(cfg.n_win)]
        for m, (row, col, val) in enumerate(mats):
            gr = gmap(row)
            sel = (gr // cfg.shard_g) == c
            grs = gr[sel] % cfg.shard_g
            t = grs // P
            rl = (grs % P).astype(np.float32)
            gcol = gmap(col[sel])
            w = gcol // WIN
            idx16 = (gcol - w * WIN).astype(np.int16)
            v = val[sel]

            bucket = t * cfg.n_win + w
            order = np.argsort(bucket, kind="stable")
            bs = bucket[order]
            cnts = np.bincount(bs, minlength=cfg.tiles * cfg.n_win)
            starts = np.zeros_like(cnts)
            starts[1:] = np.cumsum(cnts)[:-1]
            rank = np.arange(bs.size) - starts[bs]
            tt = bs // cfg.n_win
            ww = bs % cfg.n_win
            gslot = slot_base[tt, m, ww] + rank // P
            gpart = rank % P
            idx_o = idx16[order]
            rl_o = rl[order].astype(ml_dtypes.bfloat16)
            v_o = v[order].astype(ml_dtypes.bfloat16)
            for w_ in range(cfg.n_win):
                msk = ww == w_
                slot_idx[w_][gslot[msk], gpart[msk]] = idx_o[msk]
                slot_rl[w_][gslot[msk], gpart[msk]] = rl_o[msk]
                slot_v[w_][gslot[msk], gpart[msk]] = v_o[msk]

        # build gather idx image + rl/val per group (device load order)
        gidx = np.zeros((P, total_idx_cols), np.int16)
        rlv = np.zeros((P, 2, total_slots), ml_dtypes.bfloat16)
        ic = 0
        sc = 0
        for g in range(cfg.n_groups):
            for w in range(cfg.n_win):
                info = gw[(g, w)]
                a, b, e0, e1 = info["a"], info["b"], info["e0"], info["e1"]
                if b == a:
                    continue
                n_g = e1 - e0
                blk = np.zeros((n_g * GC, P), np.int16)
                lo, hi = e0 * GC, min(e1 * GC, S_w[w])
                blk[:hi - lo] = slot_idx[w][lo:hi]
                img = _slot_layout(blk.reshape(n_g, GC, P))  # [n_g, 128, 64]
                gidx[:, ic:ic + n_g * 64] = img.transpose(1, 0, 2).reshape(
                    P, n_g * 64)
                ic += n_g * 64
                rlv[:, 0, sc:sc + b - a] = slot_rl[w][a:b].T
                rlv[:, 1, sc:sc + b - a] = slot_v[w][a:b].T
                sc += b - a

        # final lookup
        sel = fowner == c
        lrow = (grow[sel] - c * cfg.shard)
        lrow = (lrow // P) * P + (lrow % P)  # local row in shard (pre-pad)
        # map to padded local position
        lrow = np.asarray(grow[sel] - c * cfg.shard, np.int64)
        ppos = pos[sel].astype(np.int32)
        cnt = lrow.size
        if cnt > cfg.bq:
            raise ValueError(f"bq too small: {cnt}")
        li = np.zeros(cfg.bq, np.int16)
        lp = np.full(cfg.bq, 2 * cfg.nb, np.int32) + np.arange(cfg.bq) % 8
        li[:cnt] = lrow.astype(np.int16)
        lp[:cnt] = ppos
        fimg = [_slot_layout(q.reshape(GC, P))
                for q in li.reshape(-1, GI)]
        fidx = np.concatenate(fimg, axis=1)
        fpos = lp.reshape(cfg.bq // P, P).T.copy()

        in_maps.append(dict(
            x0=x0, gidx=np.ascontiguousarray(gidx),
            rlv=np.ascontiguousarray(rlv),
            wt=wt, bb=bbn, iota=iota,
            fidx=np.ascontiguousarray(fidx),
            fpos=np.ascontiguousarray(fpos),
        ))
    return in_maps, aux


def postprocess(cfg: Cfg, results, aux):
    acc = np.zeros((cfg.nout, E), np.float32)
    fowner = aux["fowner"]
    for c, r in enumerate(results):
        sel = fowner == c
        acc[:2 * cfg.nb][sel] = r["out"][:2 * cfg.nb][sel]
    return acc[:cfg.nb].copy(), acc[cfg.nb:2 * cfg.nb].copy()


_CACHE = {}


def _get_nc(cfg: Cfg):
    if cfg not in _CACHE:
        _CACHE[cfg] = build_nc(cfg)
    return _CACHE[cfg]


def run(cfg, inputs, trace=False):
    nc = _get_nc(cfg)
    in_maps, aux = preprocess(cfg, **inputs)
    res = bass_utils.run_bass_kernel_spmd(
        nc, in_maps, core_ids=list(range(cfg.n_cores)), trace=trace)
    user_out, item_out = postprocess(cfg, res.results, aux)
    return (user_out, item_out), res


def kernel(user_emb, item_emb, adj_val, hp_val, W, b,
           adj_row, adj_col, hp_row, hp_col, users, items):
    inputs = dict(
        user_emb=user_emb, item_emb=item_emb, adj_val=adj_val, hp_val=hp_val,
        W=W, b=b,
        adj_row=np.asarray(adj_row).astype(np.int64),
        adj_col=np.asarray(adj_col).astype(np.int64),
        hp_row=np.asarray(hp_row).astype(np.int64),
        hp_col=np.asarray(hp_col).astype(np.int64),
        users=np.asarray(users).astype(np.int64),
        items=np.asarray(items).astype(np.int64),
    )
    cfg = make_cfg(inputs)
    (user_out, item_out), _ = run(cfg, inputs)
    return user_out, item_out
